# revision 8
# baseline (speedup 1.0000x reference)
"""Trainium2 Bass kernel for nn_CustomCLIP_11407433138213 (moe_routing).

Math (per sample b with domain n = labels[b]):
    h   = relu(x @ W1[n])                 [R]
    a   = relu(h @ W2[n])                 [D]
    f   = 0.2*a + 0.8*x                   [D]
    out = exp(ls) * (f/||f||) @ T^T       [N_TXT]

v2 strategy (vs v1 which ran all 3 experts masked for every row):
  Host sorts rows by domain and shards them so each core gets 4 row
  blocks of 512: block 0 is a "flex" block covering at most 2 domains
  (one-hot masked, 2 expert slots), blocks 1-3 are single-domain "pure"
  blocks (1 expert slot, no masks).  The expert weights a block needs
  are gathered host-side into per-block weight tensors, so the device
  program is uniform across cores while each block runs only the
  expert(s) it needs.  PE work per core drops from 744 to 520 matmuls.

  Everything is computed transposed (samples on the free dim):
    - mm1: hT[s] = W1[s]^T XT per slot (PSUM, 8 K-chunks of 128).
    - g[s] = relu(hT[s]) (pure) or relu(hT[s]) * mask[s] (flex).
    - mm2: pa[d] = sum_s W2'[s]^T g[s], W2' = 0.25*W2 (folds 0.2/0.8).
    - fp[d] = relu(pa[d]) + XT[d]  == (0.2*a + 0.8*x)/0.8 per column.
    - s = colsum(fp^2) via ones-matmul; iv = 1/sqrt(s*exp(-2 ls));
      bcast(iv) via rank-1 matmul; logits scaled after mm3.
    - mm3: logitsT[t] = TT^T fp -> * bcast(iv) -> DRAM bf16.
  mm1/mm2/mm3 operands are bf16 (full PE rate, fp32 PSUM accumulate);
  the norm chain stays fp32(r).  Host inverse-permutes the output.
  Loads are batched into a few large DMAs from host-prearranged
  layouts (one 2D copy each) to cut DMA-trigger serialization.
"""

import contextlib
import os
import sys

sys.path.insert(0, "/opt/trn_rl_repo")

import ml_dtypes
import numpy as np

import concourse.bass as bass  # noqa: F401  (registers engine types)
import concourse.mybir as mybir
import concourse.tile as tile
from concourse import bacc
from concourse.bass_utils import run_bass_kernel_spmd

# Problem constants (hardcoded per task contract).
B, D, R, ND, NT = 16384, 1024, 256, 3, 1380
NC = 8                    # cores
BPC = B // NC             # rows per core = 2048
RB = 512                  # row-block (matmul moving dim)
NB = BPC // RB            # row-blocks per core = 4
KD = D // 128             # 8 contraction chunks over D
KR = R // 128             # 2 chunks over R
MR = R // 128             # 2 M-chunks over R
NTP = 1408                # text padded to 11*128
TTI = NTP // 128          # 11 text chunks
FLEX = 0                  # block slot with 2 expert slots
PURE_T = NC * (NB - 1)    # total pure blocks = 24

F32 = mybir.dt.float32
F32R = mybir.dt.float32r
BF16 = mybir.dt.bfloat16
MM_DT = F32R if os.environ.get("KMM_DT") == "f32r" else BF16
NPDT = np.float32 if MM_DT == F32R else ml_dtypes.bfloat16


def _f32(ap):
    """View a matmul-dtype AP as plain fp32 for ACT/DVE consumption."""
    return ap.bitcast(F32) if MM_DT == F32R else ap


def build_program():
    nc = bacc.Bacc(
        "TRN2",
        target_bir_lowering=False,
        debug=False,
        enable_asserts=True,
        num_devices=NC,
    )
    xb = nc.declare_dram_parameter("xb", [NB, 128, KD * RB], MM_DT, isOutput=False)
    w1b = nc.declare_dram_parameter("w1b", [NB, 128, KD * R], MM_DT, isOutput=False)
    w2b = nc.declare_dram_parameter("w2b", [NB, 128, KR * D], MM_DT, isOutput=False)
    w1q = nc.declare_dram_parameter("w1q", [128, KD * R], MM_DT, isOutput=False)
    w2q = nc.declare_dram_parameter("w2q", [128, KR * D], MM_DT, isOutput=False)
    mk = nc.declare_dram_parameter("mk", [2, RB], F32, isOutput=False)
    tt = nc.declare_dram_parameter("tt", [128, KD * NTP], MM_DT, isOutput=False)
    sc = nc.declare_dram_parameter("sc", [1, 1], F32, isOutput=False)
    oc = nc.declare_dram_parameter("oc", [128, 1], F32R, isOutput=False)
    orow = nc.declare_dram_parameter("orow", [1, 128], F32R, isOutput=False)
    ot = nc.declare_dram_parameter("ot", [NTP, BPC], BF16, isOutput=True)

    with tile.TileContext(nc) as tc, contextlib.ExitStack() as ctx:
        cst = ctx.enter_context(tc.tile_pool(name="cst", bufs=1))
        p_xb = ctx.enter_context(tc.tile_pool(name="p_xb", bufs=2))
        p_w1 = ctx.enter_context(tc.tile_pool(name="p_w1", bufs=2))
        p_w2 = ctx.enter_context(tc.tile_pool(name="p_w2", bufs=2))
        p_g = ctx.enter_context(tc.tile_pool(name="p_g", bufs=6))
        p_fp = ctx.enter_context(tc.tile_pool(name="p_fp", bufs=16))
        p_sq = ctx.enter_context(tc.tile_pool(name="p_sq", bufs=2))
        p_acc = ctx.enter_context(tc.tile_pool(name="p_acc", bufs=4))
        p_pbs = ctx.enter_context(tc.tile_pool(name="p_pbs", bufs=2))
        p_ob = ctx.enter_context(tc.tile_pool(name="p_ob", bufs=4))
        p_nrm = ctx.enter_context(tc.tile_pool(name="p_nrm", bufs=2))

        ps_h = ctx.enter_context(tc.tile_pool(name="ps_h", bufs=2, space="PSUM"))
        ps_a = ctx.enter_context(tc.tile_pool(name="ps_a", bufs=2, space="PSUM"))
        ps_s = ctx.enter_context(tc.tile_pool(name="ps_s", bufs=1, space="PSUM"))
        ps_l = ctx.enter_context(tc.tile_pool(name="ps_l", bufs=2, space="PSUM"))

        # ---- constant tiles -------------------------------------------------
        ttt = cst.tile([128, KD * NTP], MM_DT, name="ttt", tag="ttt")
        ones_col = cst.tile([128, 1], F32R, name="ones_col", tag="ones_col")
        ones_row = cst.tile([1, 128], F32R, name="ones_row", tag="ones_row")
        sct = cst.tile([1, 1], F32, name="sct", tag="sct")
        w1qt = cst.tile([128, KD * R], MM_DT, name="w1qt", tag="w1qt")
        w2qt = cst.tile([128, KR * D], MM_DT, name="w2qt", tag="w2qt")
        mka = cst.tile([128, RB], F32, name="mka", tag="mka")
        mkb = cst.tile([128, RB], F32, name="mkb", tag="mkb")

        S = [dict() for _ in range(NB)]

        def emit_cst():
            nc.sync.dma_start(sct[:], sc[:])
            nc.sync.dma_start(ones_col[:], oc[:])
            nc.sync.dma_start(ones_row[:], orow[:])
            for i in range(4):
                w = 2 * NTP
                nc.sync.dma_start(ttt[:, i * w : (i + 1) * w], tt[:, i * w : (i + 1) * w])

        def emit_loads(b):
            w1t = p_w1.tile([128, KD * R], MM_DT, name="w1t", tag="w1t")
            nc.scalar.dma_start(w1t[:], w1b[b, :, :])
            xbig = p_xb.tile([128, KD * RB], MM_DT, name="xbig", tag="xbig")
            half = KD * RB // 2
            nc.scalar.dma_start(xbig[:, :half], xb[b, :, :half])
            nc.scalar.dma_start(xbig[:, half:], xb[b, :, half:])
            if b == FLEX:
                nc.scalar.dma_start(w1qt[:], w1q[:])
            w2t = p_w2.tile([128, KR * D], MM_DT, name="w2t", tag="w2t")
            nc.scalar.dma_start(w2t[:], w2b[b, :, :])
            if b == FLEX:
                nc.scalar.dma_start(w2qt[:], w2q[:])
                nc.scalar.dma_start(
                    mka[:],
                    mk[0, :].rearrange("(a n) -> a n", a=1).to_broadcast((128, RB)),
                )
                nc.scalar.dma_start(
                    mkb[:],
                    mk[1, :].rearrange("(a n) -> a n", a=1).to_broadcast((128, RB)),
                )
            S[b]["xbig"] = xbig
            S[b]["w1t"] = w1t
            S[b]["w2t"] = w2t

        def emit_mm1_g(b):
            xbig = S[b]["xbig"]
            slots = [(S[b]["w1t"], None)]
            if b == FLEX:
                slots = [(S[b]["w1t"], mka), (w1qt, mkb)]
            g = [[None] * MR for _ in range(len(slots))]
            for s, (wt, msk) in enumerate(slots):
                for m in range(MR):
                    ph = ps_h.tile([128, RB], F32, name="ph", tag="ph")
                    for k in range(KD):
                        nc.tensor.matmul(
                            ph[:],
                            wt[:, k * R + m * 128 : k * R + (m + 1) * 128],
                            xbig[:, k * RB : (k + 1) * RB],
                            start=(k == 0),
                            stop=(k == KD - 1),
                        )
                    gt = p_g.tile([128, RB], MM_DT, name="g", tag="g")
                    if msk is None:
                        nc.vector.tensor_scalar_max(gt[:], ph[:], 0.0)
                    else:
                        nc.vector.scalar_tensor_tensor(
                            gt[:],
                            ph[:],
                            0.0,
                            msk[:],
                            mybir.AluOpType.max,
                            mybir.AluOpType.mult,
                        )
                    g[s][m] = gt
            S[b]["g"] = g

        def emit_mm2(b):
            xbig = S[b]["xbig"]
            g = S[b]["g"]
            wts = [S[b]["w2t"]]
            if b == FLEX:
                wts = [S[b]["w2t"], w2qt]
            nmm = len(wts) * KR
            fp = []
            acc = None
            for d in range(KD):
                pa = ps_a.tile([128, RB], F32, name="pa", tag="pa")
                i = 0
                for s, wt in enumerate(wts):
                    for r in range(KR):
                        nc.tensor.matmul(
                            pa[:],
                            wt[:, r * D + d * 128 : r * D + (d + 1) * 128],
                            g[s][r][:],
                            start=(i == 0),
                            stop=(i == nmm - 1),
                        )
                        i += 1
                ft = p_fp.tile([128, RB], MM_DT, name="fp", tag="fp")
                nc.vector.scalar_tensor_tensor(
                    ft[:],
                    pa[:],
                    0.0,
                    _f32(xbig[:, d * RB : (d + 1) * RB]),
                    mybir.AluOpType.max,
                    mybir.AluOpType.add,
                )
                fp.append(ft)
                if d == 0:
                    acc = p_acc.tile([128, RB], F32, name="acc", tag="acc")
                    nc.scalar.square(acc[:], _f32(ft[:]))
                else:
                    st = p_sq.tile([128, RB], F32, name="sq", tag="sq")
                    nc.scalar.square(st[:], _f32(ft[:]))
                    if d < KD - 1:
                        nc.gpsimd.tensor_add(acc[:], acc[:], st[:])
                    else:
                        accm = p_acc.tile([128, RB], F32R, name="accm", tag="accm")
                        nc.gpsimd.tensor_add(accm[:], acc[:], st[:])
            S[b]["fp"] = fp
            S[b]["accm"] = accm

        def emit_ps_norm(b):
            accm = S[b]["accm"]
            ps = ps_s.tile([1, RB], F32, name="ps", tag="ps")
            nc.tensor.matmul(ps[:], ones_col[:], accm[:], start=True, stop=True)
            iv = p_nrm.tile([1, RB], F32R, name="iv", tag="iv")
            nc.scalar.activation(
                iv[:],
                ps[:],
                mybir.ActivationFunctionType.Abs_reciprocal_sqrt,
                scale=sct[:],
            )
            S[b]["iv"] = iv

        def emit_pb(b):
            iv = S[b]["iv"]
            pb = ps_l.tile([128, RB], F32, name="pl", tag="pl")
            nc.tensor.matmul(pb[:], ones_row[:], iv[:], start=True, stop=True)
            pbs = p_pbs.tile([128, RB], F32, name="pbs", tag="pbs")
            nc.scalar.copy(pbs[:], pb[:])
            S[b]["pbs"] = pbs

        def emit_mm3(b):
            c0 = b * RB
            fp = S[b]["fp"]
            pbs = S[b]["pbs"]
            for t_i in range(TTI):
                pl = ps_l.tile([128, RB], F32, name="pl", tag="pl")
                for k in range(KD):
                    nc.tensor.matmul(
                        pl[:],
                        ttt[:, k * NTP + t_i * 128 : k * NTP + (t_i + 1) * 128],
                        fp[k][:],
                        start=(k == 0),
                        stop=(k == KD - 1),
                    )
                ob = p_ob.tile([128, RB], BF16, name="ob", tag="ob")
                nc.vector.tensor_mul(ob[:], pl[:], pbs[:])
                nc.sync.dma_start(
                    ot[t_i * 128 : (t_i + 1) * 128, c0 : c0 + RB], ob[:]
                )
            S[b].clear()

        # ---- emission schedule (software pipelined) -------------------------
        emit_cst()
        emit_loads(0)
        emit_mm1_g(0)
        emit_loads(1)
        emit_mm2(0)
        emit_ps_norm(0)
        for b in range(NB):
            if b + 1 < NB:
                emit_mm1_g(b + 1)
            emit_pb(b)
            if b + 2 < NB:
                emit_loads(b + 2)
            if b + 1 < NB:
                emit_mm2(b + 1)
            emit_mm3(b)
            if b + 1 < NB:
                emit_ps_norm(b + 1)

    nc.compile()
    return nc


_NC_CACHE = None


def _get_program():
    global _NC_CACHE
    if _NC_CACHE is None:
        _NC_CACHE = build_program()
    return _NC_CACHE


def _pack(labels):
    """Sort rows by domain; return per-core block descriptors.

    Each core gets NB blocks of RB rows: slot FLEX may span 2 domains
    (A/B with one-hot masks), the rest are single-domain.  Returns
    per-core lists of (domA, domB, maskA, maskB, row_idx[RB]).
    """
    labels = np.asarray(labels).astype(np.int64).ravel()
    order = np.argsort(labels, kind="stable")
    counts = np.bincount(labels, minlength=ND).astype(np.int64)
    p = counts // RB
    while p.sum() > PURE_T:
        p[int(np.argmax(p))] -= 1
    assert p.sum() == PURE_T, (counts, p)

    starts = np.concatenate([[0], np.cumsum(counts)])
    pure_blocks = []
    segs = []
    for n in range(ND):
        run = order[starts[n] : starts[n + 1]]
        for i in range(int(p[n])):
            pure_blocks.append((n, run[i * RB : (i + 1) * RB]))
        segs.append((n, run[int(p[n]) * RB :]))
    # order leftover segments (small, big, small) so no flex bin spans 3 doms
    segs.sort(key=lambda s: len(s[1]))
    seg_order = [segs[0], segs[2], segs[1]]
    flat_rows = np.concatenate([s[1] for s in seg_order])
    flat_doms = np.concatenate(
        [np.full(len(s[1]), s[0], dtype=np.int64) for s in seg_order]
    )
    assert flat_rows.shape[0] == NC * RB

    cores = []
    for c in range(NC):
        rows = flat_rows[c * RB : (c + 1) * RB]
        doms = flat_doms[c * RB : (c + 1) * RB]
        dA, dB = int(doms[0]), int(doms[-1])
        mA = (doms == dA).astype(np.float32)
        mB = (1.0 - mA) if dA != dB else np.zeros(RB, dtype=np.float32)
        assert np.all((doms == dA) | (doms == dB))
        blocks = [(dA, dB, mA, mB.astype(np.float32), rows)]
        for s in range(NB - 1):
            n, prows = pure_blocks[c * (NB - 1) + s]
            blocks.append((n, n, None, None, prows))
        cores.append(blocks)
    return cores


def make_in_maps(image_features, domain_labels, W1, W2, text_features, logit_scale):
    X = np.asarray(image_features, dtype=np.float32)
    W1 = np.asarray(W1, dtype=np.float32)
    W2 = np.asarray(W2, dtype=np.float32)
    T = np.asarray(text_features, dtype=np.float32)
    ls = float(np.asarray(logit_scale))

    cores = _pack(domain_labels)

    # per-domain weight layouts for direct 2D DMA
    w1L = [
        np.ascontiguousarray(
            W1[n].reshape(KD, 128, R).transpose(1, 0, 2).reshape(128, KD * R)
        ).astype(NPDT)
        for n in range(ND)
    ]
    w2L = [
        np.ascontiguousarray(
            (0.25 * W2[n]).reshape(KR, 128, D).transpose(1, 0, 2).reshape(128, KR * D)
        ).astype(NPDT)
        for n in range(ND)
    ]
    Tpad = np.zeros((NTP, D), dtype=np.float32)
    Tpad[:NT] = T
    ttL = np.ascontiguousarray(
        Tpad.reshape(NTP, KD, 128).transpose(2, 1, 0).reshape(128, KD * NTP)
    ).astype(NPDT)
    scv = np.array([[np.exp(-2.0 * ls)]], dtype=np.float32)
    ocv = np.ones((128, 1), dtype=np.float32)
    orv = np.ones((1, 128), dtype=np.float32)

    in_maps = []
    perms = []
    for blocks in cores:
        xbs, w1s, w2s = [], [], []
        perm = []
        for dA, dB, mA, mB, rows in blocks:
            Xr = X[rows]  # [RB, D]
            xbs.append(
                Xr.reshape(RB, KD, 128).transpose(2, 1, 0).reshape(128, KD * RB)
            )
            w1s.append(w1L[dA])
            w2s.append(w2L[dA])
            perm.append(rows)
        dA0, dB0, mA0, mB0, _ = blocks[FLEX]
        in_maps.append(
            {
                "xb": np.ascontiguousarray(np.stack(xbs)).astype(NPDT),
                "w1b": np.ascontiguousarray(np.stack(w1s)),
                "w2b": np.ascontiguousarray(np.stack(w2s)),
                "w1q": w1L[dB0],
                "w2q": w2L[dB0],
                "mk": np.ascontiguousarray(np.stack([mA0, mB0])),
                "tt": ttL,
                "sc": scv,
                "oc": ocv,
                "orow": orv,
            }
        )
        perms.append(perm)
    return in_maps, perms


def kernel(image_features, domain_labels, W1, W2, text_features, logit_scale, **kw):
    in_maps, perms = make_in_maps(
        image_features, domain_labels, W1, W2, text_features, logit_scale
    )
    nc = _get_program()
    res = run_bass_kernel_spmd(nc, in_maps, list(range(NC)))

    out = np.empty((B, NT), dtype=np.float32)
    for c in range(NC):
        otc = np.asarray(res.results[c]["ot"]).astype(np.float32)
        for b in range(NB):
            out[perms[c][b], :] = otc[:NT, b * RB : (b + 1) * RB].T
    return out


# revision 10
# speedup vs baseline: 1.0644x; 1.0644x over previous
"""Trainium2 Bass kernel for nn_CustomCLIP_11407433138213 (moe_routing).

Math (per sample b with domain n = labels[b]):
    h   = relu(x @ W1[n])                 [R]
    a   = relu(h @ W2[n])                 [D]
    f   = 0.2*a + 0.8*x                   [D]
    out = exp(ls) * (f/||f||) @ T^T       [N_TXT]

v2 strategy (vs v1 which ran all 3 experts masked for every row):
  Host sorts rows by domain and shards them so each core gets 4 row
  blocks of 512: block 0 is a "flex" block covering at most 2 domains
  (one-hot masked, 2 expert slots), blocks 1-3 are single-domain "pure"
  blocks (1 expert slot, no masks).  The expert weights a block needs
  are gathered host-side into per-block weight tensors, so the device
  program is uniform across cores while each block runs only the
  expert(s) it needs.  PE work per core drops from 744 to 520 matmuls.

  Everything is computed transposed (samples on the free dim):
    - mm1: hT[s] = W1[s]^T XT per slot (PSUM, 8 K-chunks of 128).
    - g[s] = relu(hT[s]) (pure) or relu(hT[s]) * mask[s] (flex).
    - mm2: pa[d] = sum_s W2'[s]^T g[s], W2' = 0.25*W2 (folds 0.2/0.8).
    - fp[d] = relu(pa[d]) + XT[d]  == (0.2*a + 0.8*x)/0.8 per column.
    - s = colsum(fp^2) via ones-matmul; iv = 1/sqrt(s*exp(-2 ls));
      bcast(iv) via rank-1 matmul; logits scaled after mm3.
    - mm3: logitsT[t] = TT^T fp -> * bcast(iv) -> DRAM bf16.
  mm1/mm2/mm3 operands are bf16 (full PE rate, fp32 PSUM accumulate);
  the norm chain stays fp32(r).  Host inverse-permutes the output.
  Loads are batched into a few large DMAs from host-prearranged
  layouts (one 2D copy each) to cut DMA-trigger serialization.
"""

import contextlib
import os
import sys

sys.path.insert(0, "/opt/trn_rl_repo")

import ml_dtypes
import numpy as np

import concourse.bass as bass  # noqa: F401  (registers engine types)
import concourse.mybir as mybir
import concourse.tile as tile
from concourse import bacc
from concourse.bass_utils import run_bass_kernel_spmd

# Problem constants (hardcoded per task contract).
B, D, R, ND, NT = 16384, 1024, 256, 3, 1380
NC = 8                    # cores
BPC = B // NC             # rows per core = 2048
RB = 512                  # row-block (matmul moving dim)
NB = BPC // RB            # row-blocks per core = 4
KD = D // 128             # 8 contraction chunks over D
KR = R // 128             # 2 chunks over R
MR = R // 128             # 2 M-chunks over R
NTP = 1408                # text padded to 11*128
TTI = NTP // 128          # 11 text chunks
FLEX = 0                  # block slot with 2 expert slots
PURE_T = NC * (NB - 1)    # total pure blocks = 24

F32 = mybir.dt.float32
F32R = mybir.dt.float32r
BF16 = mybir.dt.bfloat16
MM_DT = F32R if os.environ.get("KMM_DT") == "f32r" else BF16
NPDT = np.float32 if MM_DT == F32R else ml_dtypes.bfloat16


def _f32(ap):
    """View a matmul-dtype AP as plain fp32 for ACT/DVE consumption."""
    return ap.bitcast(F32) if MM_DT == F32R else ap


def build_program():
    nc = bacc.Bacc(
        "TRN2",
        target_bir_lowering=False,
        debug=False,
        enable_asserts=True,
        num_devices=NC,
    )
    xb = nc.declare_dram_parameter("xb", [NB, 128, KD * RB], MM_DT, isOutput=False)
    w1b = nc.declare_dram_parameter("w1b", [NB, 128, KD * R], MM_DT, isOutput=False)
    w2b = nc.declare_dram_parameter("w2b", [NB, 128, KR * D], MM_DT, isOutput=False)
    w1q = nc.declare_dram_parameter("w1q", [128, KD * R], MM_DT, isOutput=False)
    w2q = nc.declare_dram_parameter("w2q", [128, KR * D], MM_DT, isOutput=False)
    mk = nc.declare_dram_parameter("mk", [2, RB], F32, isOutput=False)
    tt = nc.declare_dram_parameter("tt", [128, KD * NTP], MM_DT, isOutput=False)
    sc = nc.declare_dram_parameter("sc", [1, 1], F32, isOutput=False)
    oc = nc.declare_dram_parameter("oc", [128, 1], F32R, isOutput=False)
    orow = nc.declare_dram_parameter("orow", [1, 128], F32R, isOutput=False)
    ot = nc.declare_dram_parameter("ot", [NTP, BPC], BF16, isOutput=True)

    with tile.TileContext(nc) as tc, contextlib.ExitStack() as ctx:
        cst = ctx.enter_context(tc.tile_pool(name="cst", bufs=1))
        p_xb = ctx.enter_context(tc.tile_pool(name="p_xb", bufs=2))
        p_w1 = ctx.enter_context(tc.tile_pool(name="p_w1", bufs=2))
        p_w2 = ctx.enter_context(tc.tile_pool(name="p_w2", bufs=2))
        p_g = ctx.enter_context(tc.tile_pool(name="p_g", bufs=6))
        p_fp = ctx.enter_context(tc.tile_pool(name="p_fp", bufs=16))
        p_sq = ctx.enter_context(tc.tile_pool(name="p_sq", bufs=2))
        p_acc = ctx.enter_context(tc.tile_pool(name="p_acc", bufs=4))
        p_pbs = ctx.enter_context(tc.tile_pool(name="p_pbs", bufs=2))
        p_ob = ctx.enter_context(tc.tile_pool(name="p_ob", bufs=4))
        p_nrm = ctx.enter_context(tc.tile_pool(name="p_nrm", bufs=2))

        ps_h = ctx.enter_context(tc.tile_pool(name="ps_h", bufs=2, space="PSUM"))
        ps_a = ctx.enter_context(tc.tile_pool(name="ps_a", bufs=2, space="PSUM"))
        ps_s = ctx.enter_context(tc.tile_pool(name="ps_s", bufs=1, space="PSUM"))
        ps_l = ctx.enter_context(tc.tile_pool(name="ps_l", bufs=2, space="PSUM"))

        # ---- constant tiles -------------------------------------------------
        ttt = cst.tile([128, KD * NTP], MM_DT, name="ttt", tag="ttt")
        ones_col = cst.tile([128, 1], F32R, name="ones_col", tag="ones_col")
        ones_row = cst.tile([1, 128], F32R, name="ones_row", tag="ones_row")
        sct = cst.tile([1, 1], F32, name="sct", tag="sct")
        w1qt = cst.tile([128, KD * R], MM_DT, name="w1qt", tag="w1qt")
        w2qt = cst.tile([128, KR * D], MM_DT, name="w2qt", tag="w2qt")
        mka = cst.tile([128, RB], F32, name="mka", tag="mka")
        mkb = cst.tile([128, RB], F32, name="mkb", tag="mkb")

        S = [dict() for _ in range(NB)]

        def emit_cst():
            nc.sync.dma_start(sct[:], sc[:])
            nc.sync.dma_start(ones_col[:], oc[:])
            nc.sync.dma_start(ones_row[:], orow[:])

        def emit_tt():
            # on the scalar queue AFTER block-0/1 loads so the big text
            # matrix doesn't compete with the latency-critical prologue
            for i in range(4):
                w = 2 * NTP
                nc.scalar.dma_start(ttt[:, i * w : (i + 1) * w], tt[:, i * w : (i + 1) * w])

        def emit_loads(b):
            w1t = p_w1.tile([128, KD * R], MM_DT, name="w1t", tag="w1t")
            nc.scalar.dma_start(w1t[:], w1b[b, :, :])
            xbig = p_xb.tile([128, KD * RB], MM_DT, name="xbig", tag="xbig")
            half = KD * RB // 2
            nc.scalar.dma_start(xbig[:, :half], xb[b, :, :half])
            nc.scalar.dma_start(xbig[:, half:], xb[b, :, half:])
            if b == FLEX:
                nc.scalar.dma_start(w1qt[:], w1q[:])
            w2t = p_w2.tile([128, KR * D], MM_DT, name="w2t", tag="w2t")
            nc.scalar.dma_start(w2t[:], w2b[b, :, :])
            if b == FLEX:
                nc.scalar.dma_start(w2qt[:], w2q[:])
                nc.scalar.dma_start(
                    mka[:],
                    mk[0, :].rearrange("(a n) -> a n", a=1).to_broadcast((128, RB)),
                )
                nc.scalar.dma_start(
                    mkb[:],
                    mk[1, :].rearrange("(a n) -> a n", a=1).to_broadcast((128, RB)),
                )
            S[b]["xbig"] = xbig
            S[b]["w1t"] = w1t
            S[b]["w2t"] = w2t

        def emit_mm1_g(b):
            xbig = S[b]["xbig"]
            slots = [(S[b]["w1t"], None)]
            if b == FLEX:
                slots = [(S[b]["w1t"], mka), (w1qt, mkb)]
            g = [[None] * MR for _ in range(len(slots))]
            for s, (wt, msk) in enumerate(slots):
                for m in range(MR):
                    ph = ps_h.tile([128, RB], F32, name="ph", tag="ph")
                    for k in range(KD):
                        nc.tensor.matmul(
                            ph[:],
                            wt[:, k * R + m * 128 : k * R + (m + 1) * 128],
                            xbig[:, k * RB : (k + 1) * RB],
                            start=(k == 0),
                            stop=(k == KD - 1),
                        )
                    gt = p_g.tile([128, RB], MM_DT, name="g", tag="g")
                    if msk is None:
                        nc.vector.tensor_scalar_max(gt[:], ph[:], 0.0)
                    else:
                        nc.vector.scalar_tensor_tensor(
                            gt[:],
                            ph[:],
                            0.0,
                            msk[:],
                            mybir.AluOpType.max,
                            mybir.AluOpType.mult,
                        )
                    g[s][m] = gt
            S[b]["g"] = g

        def emit_mm2(b):
            xbig = S[b]["xbig"]
            g = S[b]["g"]
            wts = [S[b]["w2t"]]
            if b == FLEX:
                wts = [S[b]["w2t"], w2qt]
            nmm = len(wts) * KR
            fp = []
            acc = None
            for d in range(KD):
                pa = ps_a.tile([128, RB], F32, name="pa", tag="pa")
                i = 0
                for s, wt in enumerate(wts):
                    for r in range(KR):
                        nc.tensor.matmul(
                            pa[:],
                            wt[:, r * D + d * 128 : r * D + (d + 1) * 128],
                            g[s][r][:],
                            start=(i == 0),
                            stop=(i == nmm - 1),
                        )
                        i += 1
                ft = p_fp.tile([128, RB], MM_DT, name="fp", tag="fp")
                nc.vector.scalar_tensor_tensor(
                    ft[:],
                    pa[:],
                    0.0,
                    _f32(xbig[:, d * RB : (d + 1) * RB]),
                    mybir.AluOpType.max,
                    mybir.AluOpType.add,
                )
                fp.append(ft)
                if d == 0:
                    acc = p_acc.tile([128, RB], F32, name="acc", tag="acc")
                    nc.scalar.square(acc[:], _f32(ft[:]))
                else:
                    st = p_sq.tile([128, RB], F32, name="sq", tag="sq")
                    nc.scalar.square(st[:], _f32(ft[:]))
                    if d < KD - 1:
                        nc.gpsimd.tensor_add(acc[:], acc[:], st[:])
                    else:
                        accm = p_acc.tile([128, RB], F32R, name="accm", tag="accm")
                        nc.gpsimd.tensor_add(accm[:], acc[:], st[:])
            S[b]["fp"] = fp
            S[b]["accm"] = accm

        def emit_ps_norm(b):
            accm = S[b]["accm"]
            ps = ps_s.tile([1, RB], F32, name="ps", tag="ps")
            nc.tensor.matmul(ps[:], ones_col[:], accm[:], start=True, stop=True)
            iv = p_nrm.tile([1, RB], F32R, name="iv", tag="iv")
            nc.scalar.activation(
                iv[:],
                ps[:],
                mybir.ActivationFunctionType.Abs_reciprocal_sqrt,
                scale=sct[:],
            )
            S[b]["iv"] = iv

        def emit_pb(b):
            iv = S[b]["iv"]
            pb = ps_l.tile([128, RB], F32, name="pl", tag="pl")
            nc.tensor.matmul(pb[:], ones_row[:], iv[:], start=True, stop=True)
            pbs = p_pbs.tile([128, RB], F32, name="pbs", tag="pbs")
            nc.scalar.copy(pbs[:], pb[:])
            S[b]["pbs"] = pbs

        def emit_mm3(b):
            c0 = b * RB
            fp = S[b]["fp"]
            pbs = S[b]["pbs"]
            for t_i in range(TTI):
                pl = ps_l.tile([128, RB], F32, name="pl", tag="pl")
                for k in range(KD):
                    nc.tensor.matmul(
                        pl[:],
                        ttt[:, k * NTP + t_i * 128 : k * NTP + (t_i + 1) * 128],
                        fp[k][:],
                        start=(k == 0),
                        stop=(k == KD - 1),
                    )
                ob = p_ob.tile([128, RB], BF16, name="ob", tag="ob")
                nc.vector.tensor_mul(ob[:], pl[:], pbs[:])
                nc.sync.dma_start(
                    ot[t_i * 128 : (t_i + 1) * 128, c0 : c0 + RB], ob[:]
                )
            S[b].clear()

        # ---- emission schedule (software pipelined) -------------------------
        emit_cst()
        emit_loads(0)
        emit_mm1_g(0)
        emit_loads(1)
        emit_tt()
        emit_mm2(0)
        emit_ps_norm(0)
        for b in range(NB):
            if b + 1 < NB:
                emit_mm1_g(b + 1)
            emit_pb(b)
            if b + 2 < NB:
                emit_loads(b + 2)
            if b + 1 < NB:
                emit_mm2(b + 1)
            emit_mm3(b)
            if b + 1 < NB:
                emit_ps_norm(b + 1)

    nc.compile()
    return nc


_NC_CACHE = None


def _get_program():
    global _NC_CACHE
    if _NC_CACHE is None:
        _NC_CACHE = build_program()
    return _NC_CACHE


def _pack(labels):
    """Sort rows by domain; return per-core block descriptors.

    Each core gets NB blocks of RB rows: slot FLEX may span 2 domains
    (A/B with one-hot masks), the rest are single-domain.  Returns
    per-core lists of (domA, domB, maskA, maskB, row_idx[RB]).
    """
    labels = np.asarray(labels).astype(np.int64).ravel()
    order = np.argsort(labels, kind="stable")
    counts = np.bincount(labels, minlength=ND).astype(np.int64)
    p = counts // RB
    while p.sum() > PURE_T:
        p[int(np.argmax(p))] -= 1
    assert p.sum() == PURE_T, (counts, p)

    starts = np.concatenate([[0], np.cumsum(counts)])
    pure_blocks = []
    segs = []
    for n in range(ND):
        run = order[starts[n] : starts[n + 1]]
        for i in range(int(p[n])):
            pure_blocks.append((n, run[i * RB : (i + 1) * RB]))
        segs.append((n, run[int(p[n]) * RB :]))
    # order leftover segments (small, big, small) so no flex bin spans 3 doms
    segs.sort(key=lambda s: len(s[1]))
    seg_order = [segs[0], segs[2], segs[1]]
    flat_rows = np.concatenate([s[1] for s in seg_order])
    flat_doms = np.concatenate(
        [np.full(len(s[1]), s[0], dtype=np.int64) for s in seg_order]
    )
    assert flat_rows.shape[0] == NC * RB

    cores = []
    for c in range(NC):
        rows = flat_rows[c * RB : (c + 1) * RB]
        doms = flat_doms[c * RB : (c + 1) * RB]
        dA, dB = int(doms[0]), int(doms[-1])
        mA = (doms == dA).astype(np.float32)
        mB = (1.0 - mA) if dA != dB else np.zeros(RB, dtype=np.float32)
        assert np.all((doms == dA) | (doms == dB))
        blocks = [(dA, dB, mA, mB.astype(np.float32), rows)]
        for s in range(NB - 1):
            n, prows = pure_blocks[c * (NB - 1) + s]
            blocks.append((n, n, None, None, prows))
        cores.append(blocks)
    return cores


def make_in_maps(image_features, domain_labels, W1, W2, text_features, logit_scale):
    X = np.asarray(image_features, dtype=np.float32)
    W1 = np.asarray(W1, dtype=np.float32)
    W2 = np.asarray(W2, dtype=np.float32)
    T = np.asarray(text_features, dtype=np.float32)
    ls = float(np.asarray(logit_scale))

    cores = _pack(domain_labels)

    # per-domain weight layouts for direct 2D DMA
    w1L = [
        np.ascontiguousarray(
            W1[n].reshape(KD, 128, R).transpose(1, 0, 2).reshape(128, KD * R)
        ).astype(NPDT)
        for n in range(ND)
    ]
    w2L = [
        np.ascontiguousarray(
            (0.25 * W2[n]).reshape(KR, 128, D).transpose(1, 0, 2).reshape(128, KR * D)
        ).astype(NPDT)
        for n in range(ND)
    ]
    Tpad = np.zeros((NTP, D), dtype=np.float32)
    Tpad[:NT] = T
    ttL = np.ascontiguousarray(
        Tpad.reshape(NTP, KD, 128).transpose(2, 1, 0).reshape(128, KD * NTP)
    ).astype(NPDT)
    scv = np.array([[np.exp(-2.0 * ls)]], dtype=np.float32)
    ocv = np.ones((128, 1), dtype=np.float32)
    orv = np.ones((1, 128), dtype=np.float32)

    in_maps = []
    perms = []
    for blocks in cores:
        xbs, w1s, w2s = [], [], []
        perm = []
        for dA, dB, mA, mB, rows in blocks:
            Xr = X[rows]  # [RB, D]
            xbs.append(
                Xr.reshape(RB, KD, 128).transpose(2, 1, 0).reshape(128, KD * RB)
            )
            w1s.append(w1L[dA])
            w2s.append(w2L[dA])
            perm.append(rows)
        dA0, dB0, mA0, mB0, _ = blocks[FLEX]
        in_maps.append(
            {
                "xb": np.ascontiguousarray(np.stack(xbs)).astype(NPDT),
                "w1b": np.ascontiguousarray(np.stack(w1s)),
                "w2b": np.ascontiguousarray(np.stack(w2s)),
                "w1q": w1L[dB0],
                "w2q": w2L[dB0],
                "mk": np.ascontiguousarray(np.stack([mA0, mB0])),
                "tt": ttL,
                "sc": scv,
                "oc": ocv,
                "orow": orv,
            }
        )
        perms.append(perm)
    return in_maps, perms


def kernel(image_features, domain_labels, W1, W2, text_features, logit_scale, **kw):
    in_maps, perms = make_in_maps(
        image_features, domain_labels, W1, W2, text_features, logit_scale
    )
    nc = _get_program()
    res = run_bass_kernel_spmd(nc, in_maps, list(range(NC)))

    out = np.empty((B, NT), dtype=np.float32)
    for c in range(NC):
        otc = np.asarray(res.results[c]["ot"]).astype(np.float32)
        for b in range(NB):
            out[perms[c][b], :] = otc[:NT, b * RB : (b + 1) * RB].T
    return out


# revision 19
# speedup vs baseline: 1.1046x; 1.0378x over previous
"""Trainium2 Bass kernel for nn_CustomCLIP_11407433138213 (moe_routing).

Math (per sample b with domain n = labels[b]):
    h   = relu(x @ W1[n])                 [R]
    a   = relu(h @ W2[n])                 [D]
    f   = 0.2*a + 0.8*x                   [D]
    out = exp(ls) * (f/||f||) @ T^T       [N_TXT]

v2 strategy (vs v1 which ran all 3 experts masked for every row):
  Host sorts rows by domain and shards them so each core gets 4 row
  blocks of 512: block 0 is a "flex" block covering at most 2 domains
  (one-hot masked, 2 expert slots), blocks 1-3 are single-domain "pure"
  blocks (1 expert slot, no masks).  The expert weights a block needs
  are gathered host-side into per-block weight tensors, so the device
  program is uniform across cores while each block runs only the
  expert(s) it needs.  PE work per core drops from 744 to 520 matmuls.

  Everything is computed transposed (samples on the free dim):
    - mm1: hT[s] = W1[s]^T XT per slot (PSUM, 8 K-chunks of 128).
    - g[s] = relu(hT[s]) (pure) or relu(hT[s]) * mask[s] (flex).
    - mm2: pa[d] = sum_s W2'[s]^T g[s], W2' = 0.25*W2 (folds 0.2/0.8).
    - fp[d] = relu(pa[d]) + XT[d]  == (0.2*a + 0.8*x)/0.8 per column.
    - s = colsum(fp^2) via ones-matmul; iv = 1/sqrt(s*exp(-2 ls));
      bcast(iv) via rank-1 matmul; logits scaled after mm3.
    - mm3: logitsT[t] = TT^T fp -> * bcast(iv) -> DRAM bf16.
  mm1/mm2/mm3 operands are bf16 (full PE rate, fp32 PSUM accumulate);
  the norm chain stays fp32(r).  Host inverse-permutes the output.
  Loads are batched into a few large DMAs from host-prearranged
  layouts (one 2D copy each) to cut DMA-trigger serialization.
"""

import contextlib
import os
import sys

sys.path.insert(0, "/opt/trn_rl_repo")

import ml_dtypes
import numpy as np

import concourse.bass as bass  # noqa: F401  (registers engine types)
import concourse.mybir as mybir
import concourse.tile as tile
from concourse import bacc
from concourse.bass_utils import run_bass_kernel_spmd

# Problem constants (hardcoded per task contract).
B, D, R, ND, NT = 16384, 1024, 256, 3, 1380
NC = 8                    # cores
BPC = B // NC             # rows per core = 2048
RB = 512                  # row-block (matmul moving dim)
NB = BPC // RB            # row-blocks per core = 4
KD = D // 128             # 8 contraction chunks over D
KR = R // 128             # 2 chunks over R
MR = R // 128             # 2 M-chunks over R
NTP = 1408                # text padded to 11*128
TTI = NTP // 128          # 11 text chunks
FLEX = 0                  # block slot with 2 expert slots
PURE_T = NC * (NB - 1)    # total pure blocks = 24

F32 = mybir.dt.float32
F32R = mybir.dt.float32r
BF16 = mybir.dt.bfloat16
F8 = mybir.dt.float8e4
MM_DT = BF16
NPDT = ml_dtypes.bfloat16
NPF8 = ml_dtypes.float8_e4m3fn
# fp8 scale plan: W1*64, W2*64 -> psum = 4096*(h@W2); residual x *16384
# makes fp_scaled = 16384*(0.25*relu(a) + x); the norm cancels the scale.
W_SCALE = 64.0
XR_SCALE = 16384.0


def _f32(ap):
    """View a matmul-dtype AP as plain fp32 for ACT/DVE consumption."""
    return ap.bitcast(F32) if MM_DT == F32R else ap


def _dr(ap):
    """Split an AP's free dim into the 2 k-tiles DoubleRow matmul expects."""
    return ap.rearrange("p (k n) -> p k n", k=2)


def build_program():
    nc = bacc.Bacc(
        "TRN2",
        target_bir_lowering=False,
        debug=False,
        enable_asserts=True,
        num_devices=NC,
    )
    xb = nc.declare_dram_parameter("xb", [NB, 128, KD * RB], F8, isOutput=False)
    xr = nc.declare_dram_parameter("xr", [NB, 128, KD * RB], BF16, isOutput=False)
    w1b = nc.declare_dram_parameter("w1b", [NB, 128, KD * R], F8, isOutput=False)
    w2b = nc.declare_dram_parameter("w2b", [NB, 128, KR * D], F8, isOutput=False)
    w1q = nc.declare_dram_parameter("w1q", [128, KD * R], F8, isOutput=False)
    w2q = nc.declare_dram_parameter("w2q", [128, KR * D], F8, isOutput=False)
    mk = nc.declare_dram_parameter("mk", [2, RB], F32, isOutput=False)
    tt = nc.declare_dram_parameter("tt", [128, KD * NTP], MM_DT, isOutput=False)
    sc = nc.declare_dram_parameter("sc", [1, 1], F32, isOutput=False)
    oc = nc.declare_dram_parameter("oc", [128, 1], F32R, isOutput=False)
    orow = nc.declare_dram_parameter("orow", [1, 128], F32R, isOutput=False)
    ot = nc.declare_dram_parameter("ot", [NTP, BPC], BF16, isOutput=True)

    with tile.TileContext(nc) as tc, contextlib.ExitStack() as ctx:
        cst = ctx.enter_context(tc.tile_pool(name="cst", bufs=1))
        p_xb = ctx.enter_context(tc.tile_pool(name="p_xb", bufs=2))
        p_xr = ctx.enter_context(tc.tile_pool(name="p_xr", bufs=2))
        p_w1 = ctx.enter_context(tc.tile_pool(name="p_w1", bufs=2))
        p_w2 = ctx.enter_context(tc.tile_pool(name="p_w2", bufs=2))
        p_g = ctx.enter_context(tc.tile_pool(name="p_g", bufs=6))
        p_fp = ctx.enter_context(tc.tile_pool(name="p_fp", bufs=16))
        p_sq = ctx.enter_context(tc.tile_pool(name="p_sq", bufs=2))
        p_acc = ctx.enter_context(tc.tile_pool(name="p_acc", bufs=4))
        p_pbs = ctx.enter_context(tc.tile_pool(name="p_pbs", bufs=2))
        p_ob = ctx.enter_context(tc.tile_pool(name="p_ob", bufs=4))
        p_nrm = ctx.enter_context(tc.tile_pool(name="p_nrm", bufs=2))

        ps_h = ctx.enter_context(tc.tile_pool(name="ps_h", bufs=2, space="PSUM"))
        ps_a = ctx.enter_context(tc.tile_pool(name="ps_a", bufs=2, space="PSUM"))
        ps_s = ctx.enter_context(tc.tile_pool(name="ps_s", bufs=1, space="PSUM"))
        ps_l = ctx.enter_context(tc.tile_pool(name="ps_l", bufs=2, space="PSUM"))

        # ---- constant tiles -------------------------------------------------
        ttt = cst.tile([128, KD * NTP], MM_DT, name="ttt", tag="ttt")
        ones_col = cst.tile([128, 1], F32R, name="ones_col", tag="ones_col")
        ones_row = cst.tile([1, 128], F32R, name="ones_row", tag="ones_row")
        sct = cst.tile([1, 1], F32, name="sct", tag="sct")
        w1qt = cst.tile([128, KD * R], F8, name="w1qt", tag="w1qt")
        w2qt = cst.tile([128, KR * D], F8, name="w2qt", tag="w2qt")
        mka = cst.tile([128, RB], F32, name="mka", tag="mka")
        mkb = cst.tile([128, RB], F32, name="mkb", tag="mkb")

        S = [dict() for _ in range(NB)]

        def emit_cst():
            nc.sync.dma_start(sct[:], sc[:])
            nc.sync.dma_start(ones_col[:], oc[:])
            nc.sync.dma_start(ones_row[:], orow[:])

        def emit_tt():
            # on the scalar queue AFTER block-0/1 loads so the big text
            # matrix doesn't compete with the latency-critical prologue
            for i in range(4):
                w = 2 * NTP
                nc.scalar.dma_start(ttt[:, i * w : (i + 1) * w], tt[:, i * w : (i + 1) * w])

        def emit_loads(b):
            w1t = p_w1.tile([128, KD * R], F8, name="w1t", tag="w1t")
            nc.scalar.dma_start(w1t[:], w1b[b, :, :])
            xbig = p_xb.tile([128, KD * RB], F8, name="xbig", tag="xbig")
            nc.scalar.dma_start(xbig[:], xb[b, :, :])
            if b == FLEX:
                nc.scalar.dma_start(w1qt[:], w1q[:])
            w2t = p_w2.tile([128, KR * D], F8, name="w2t", tag="w2t")
            nc.scalar.dma_start(w2t[:], w2b[b, :, :])
            if b == FLEX:
                nc.scalar.dma_start(w2qt[:], w2q[:])
                nc.scalar.dma_start(
                    mka[:],
                    mk[0, :].rearrange("(a n) -> a n", a=1).to_broadcast((128, RB)),
                )
                nc.scalar.dma_start(
                    mkb[:],
                    mk[1, :].rearrange("(a n) -> a n", a=1).to_broadcast((128, RB)),
                )
            xres = p_xr.tile([128, KD * RB], BF16, name="xres", tag="xres")
            half = KD * RB // 2
            nc.scalar.dma_start(xres[:, :half], xr[b, :, :half])
            nc.scalar.dma_start(xres[:, half:], xr[b, :, half:])
            S[b]["xbig"] = xbig
            S[b]["xres"] = xres
            S[b]["w1t"] = w1t
            S[b]["w2t"] = w2t

        DRM = mybir.MatmulPerfMode.DoubleRow

        def emit_mm1_g(b):
            xbig = S[b]["xbig"]
            slots = [(S[b]["w1t"], None)]
            if b == FLEX:
                slots = [(S[b]["w1t"], mka), (w1qt, mkb)]
            g = []
            for s, (wt, msk) in enumerate(slots):
                # one g tile per slot holding both r-chunks side by side
                # (the two k-tiles DoubleRow mm2 consumes)
                gt = p_g.tile([128, 2 * RB], F8, name="g", tag="g")
                for m in range(MR):
                    ph = ps_h.tile([128, RB], F32, name="ph", tag="ph")
                    for i in range(KD // 2):
                        nc.tensor.matmul(
                            ph[:],
                            _dr(wt[:, i * 2 * R + m * 2 * 128 : i * 2 * R + (m + 1) * 2 * 128]),
                            _dr(xbig[:, 2 * i * RB : (2 * i + 2) * RB]),
                            start=(i == 0),
                            stop=(i == KD // 2 - 1),
                            perf_mode=DRM,
                        )
                    gd = gt[:, m * RB : (m + 1) * RB]
                    if msk is None:
                        nc.vector.tensor_scalar_max(gd, ph[:], 0.0)
                    else:
                        nc.vector.scalar_tensor_tensor(
                            gd,
                            ph[:],
                            0.0,
                            msk[:],
                            mybir.AluOpType.max,
                            mybir.AluOpType.mult,
                        )
                g.append(gt)
            S[b]["g"] = g

        def emit_mm2(b):
            xres = S[b]["xres"]
            g = S[b]["g"]
            wts = [S[b]["w2t"]]
            if b == FLEX:
                wts = [S[b]["w2t"], w2qt]
            nmm = len(wts)
            fp = []
            acc = None
            for d in range(KD):
                pa = ps_a.tile([128, RB], F32, name="pa", tag="pa")
                for s, wt in enumerate(wts):
                    nc.tensor.matmul(
                        pa[:],
                        _dr(wt[:, d * 256 : (d + 1) * 256]),
                        _dr(g[s][:]),
                        start=(s == 0),
                        stop=(s == nmm - 1),
                        perf_mode=DRM,
                    )
                ft = p_fp.tile([128, RB], MM_DT, name="fp", tag="fp")
                nc.vector.scalar_tensor_tensor(
                    ft[:],
                    pa[:],
                    0.0,
                    xres[:, d * RB : (d + 1) * RB],
                    mybir.AluOpType.max,
                    mybir.AluOpType.add,
                )
                fp.append(ft)
                if d == 0:
                    acc = p_acc.tile([128, RB], F32, name="acc", tag="acc")
                    nc.scalar.square(acc[:], _f32(ft[:]))
                else:
                    st = p_sq.tile([128, RB], F32, name="sq", tag="sq")
                    nc.scalar.square(st[:], _f32(ft[:]))
                    if d < KD - 1:
                        nc.gpsimd.tensor_add(acc[:], acc[:], st[:])
                    else:
                        accm = p_acc.tile([128, RB], F32R, name="accm", tag="accm")
                        nc.gpsimd.tensor_add(accm[:], acc[:], st[:])
            S[b]["fp"] = fp
            S[b]["accm"] = accm

        def emit_ps_norm(b):
            accm = S[b]["accm"]
            ps = ps_s.tile([1, RB], F32, name="ps", tag="ps")
            nc.tensor.matmul(ps[:], ones_col[:], accm[:], start=True, stop=True)
            iv = p_nrm.tile([1, RB], F32R, name="iv", tag="iv")
            nc.scalar.activation(
                iv[:],
                ps[:],
                mybir.ActivationFunctionType.Abs_reciprocal_sqrt,
                scale=sct[:],
            )
            S[b]["iv"] = iv

        def emit_pb(b):
            iv = S[b]["iv"]
            pb = ps_l.tile([128, RB], F32, name="pl", tag="pl")
            nc.tensor.matmul(pb[:], ones_row[:], iv[:], start=True, stop=True)
            pbs = p_pbs.tile([128, RB], F32, name="pbs", tag="pbs")
            nc.scalar.copy(pbs[:], pb[:])
            S[b]["pbs"] = pbs

        def emit_mm3(b):
            c0 = b * RB
            fp = S[b]["fp"]
            pbs = S[b]["pbs"]
            for t_i in range(TTI):
                pl = ps_l.tile([128, RB], F32, name="pl", tag="pl")
                for k in range(KD):
                    nc.tensor.matmul(
                        pl[:],
                        ttt[:, k * NTP + t_i * 128 : k * NTP + (t_i + 1) * 128],
                        fp[k][:],
                        start=(k == 0),
                        stop=(k == KD - 1),
                    )
                ob = p_ob.tile([128, RB], BF16, name="ob", tag="ob")
                nc.vector.tensor_mul(ob[:], pl[:], pbs[:])
                nc.sync.dma_start(
                    ot[t_i * 128 : (t_i + 1) * 128, c0 : c0 + RB], ob[:]
                )
            S[b].clear()

        # ---- emission schedule (software pipelined) -------------------------
        emit_cst()
        emit_loads(0)
        emit_mm1_g(0)
        emit_loads(1)
        emit_tt()
        emit_mm2(0)
        emit_ps_norm(0)
        for b in range(NB):
            if b + 1 < NB:
                emit_mm1_g(b + 1)
            emit_pb(b)
            if b + 2 < NB:
                emit_loads(b + 2)
            if b + 1 < NB:
                emit_mm2(b + 1)
            emit_mm3(b)
            if b + 1 < NB:
                emit_ps_norm(b + 1)

    nc.compile()
    return nc


_NC_CACHE = None


def _get_program():
    global _NC_CACHE
    if _NC_CACHE is None:
        _NC_CACHE = build_program()
    return _NC_CACHE


def _pack(labels):
    """Sort rows by domain; return per-core block descriptors.

    Each core gets NB blocks of RB rows: slot FLEX may span 2 domains
    (A/B with one-hot masks), the rest are single-domain.  Returns
    per-core lists of (domA, domB, maskA, maskB, row_idx[RB]).
    """
    labels = np.asarray(labels).astype(np.int64).ravel()
    order = np.argsort(labels, kind="stable")
    counts = np.bincount(labels, minlength=ND).astype(np.int64)
    p = counts // RB
    while p.sum() > PURE_T:
        p[int(np.argmax(p))] -= 1
    assert p.sum() == PURE_T, (counts, p)

    starts = np.concatenate([[0], np.cumsum(counts)])
    pure_blocks = []
    segs = []
    for n in range(ND):
        run = order[starts[n] : starts[n + 1]]
        for i in range(int(p[n])):
            pure_blocks.append((n, run[i * RB : (i + 1) * RB]))
        segs.append((n, run[int(p[n]) * RB :]))
    # order leftover segments (small, big, small) so no flex bin spans 3 doms
    segs.sort(key=lambda s: len(s[1]))
    seg_order = [segs[0], segs[2], segs[1]]
    flat_rows = np.concatenate([s[1] for s in seg_order])
    flat_doms = np.concatenate(
        [np.full(len(s[1]), s[0], dtype=np.int64) for s in seg_order]
    )
    assert flat_rows.shape[0] == NC * RB

    cores = []
    for c in range(NC):
        rows = flat_rows[c * RB : (c + 1) * RB]
        doms = flat_doms[c * RB : (c + 1) * RB]
        dA, dB = int(doms[0]), int(doms[-1])
        mA = (doms == dA).astype(np.float32)
        mB = (1.0 - mA) if dA != dB else np.zeros(RB, dtype=np.float32)
        assert np.all((doms == dA) | (doms == dB))
        blocks = [(dA, dB, mA, mB.astype(np.float32), rows)]
        for s in range(NB - 1):
            n, prows = pure_blocks[c * (NB - 1) + s]
            blocks.append((n, n, None, None, prows))
        cores.append(blocks)
    return cores


def make_in_maps(image_features, domain_labels, W1, W2, text_features, logit_scale):
    X = np.asarray(image_features, dtype=np.float32)
    W1 = np.asarray(W1, dtype=np.float32)
    W2 = np.asarray(W2, dtype=np.float32)
    T = np.asarray(text_features, dtype=np.float32)
    ls = float(np.asarray(logit_scale))

    cores = _pack(domain_labels)

    # per-domain weight layouts for direct 2D DMA (fp8 DoubleRow order)
    # w1: [p, i*512 + m*256 + j*128 + mc] = 64*W1[(2i+j)*128+p, m*128+mc]
    w1L = [
        np.ascontiguousarray(
            (W_SCALE * W1[n])
            .reshape(KD // 2, 2, 128, MR, 128)
            .transpose(2, 0, 3, 1, 4)
            .reshape(128, KD * R)
        ).astype(NPF8)
        for n in range(ND)
    ]
    # w2: [p, d*256 + j*128 + dc] = 64*W2[j*128+p, d*128+dc]
    w2L = [
        np.ascontiguousarray(
            (W_SCALE * W2[n])
            .reshape(KR, 128, KD, 128)
            .transpose(1, 2, 0, 3)
            .reshape(128, KR * D)
        ).astype(NPF8)
        for n in range(ND)
    ]
    Tpad = np.zeros((NTP, D), dtype=np.float32)
    Tpad[:NT] = T
    ttL = np.ascontiguousarray(
        Tpad.reshape(NTP, KD, 128).transpose(2, 1, 0).reshape(128, KD * NTP)
    ).astype(NPDT)
    scv = np.array([[np.exp(-2.0 * ls)]], dtype=np.float32)
    ocv = np.ones((128, 1), dtype=np.float32)
    orv = np.ones((1, 128), dtype=np.float32)

    in_maps = []
    perms = []
    for blocks in cores:
        xbs, w1s, w2s = [], [], []
        perm = []
        for dA, dB, mA, mB, rows in blocks:
            Xr = X[rows]  # [RB, D]
            xbs.append(
                Xr.reshape(RB, KD, 128).transpose(2, 1, 0).reshape(128, KD * RB)
            )
            w1s.append(w1L[dA])
            w2s.append(w2L[dA])
            perm.append(rows)
        dA0, dB0, mA0, mB0, _ = blocks[FLEX]
        xbf = np.ascontiguousarray(np.stack(xbs))
        in_maps.append(
            {
                "xb": xbf.astype(NPF8),
                "xr": (XR_SCALE * xbf).astype(NPDT),
                "w1b": np.ascontiguousarray(np.stack(w1s)),
                "w2b": np.ascontiguousarray(np.stack(w2s)),
                "w1q": w1L[dB0],
                "w2q": w2L[dB0],
                "mk": np.ascontiguousarray(np.stack([mA0, mB0])),
                "tt": ttL,
                "sc": scv,
                "oc": ocv,
                "orow": orv,
            }
        )
        perms.append(perm)
    return in_maps, perms


def kernel(image_features, domain_labels, W1, W2, text_features, logit_scale, **kw):
    in_maps, perms = make_in_maps(
        image_features, domain_labels, W1, W2, text_features, logit_scale
    )
    nc = _get_program()
    res = run_bass_kernel_spmd(nc, in_maps, list(range(NC)))

    out = np.empty((B, NT), dtype=np.float32)
    for c in range(NC):
        otc = np.asarray(res.results[c]["ot"]).astype(np.float32)
        for b in range(NB):
            out[perms[c][b], :] = otc[:NT, b * RB : (b + 1) * RB].T
    return out


# revision 24
# speedup vs baseline: 1.1149x; 1.0093x over previous
"""Trainium2 Bass kernel for nn_CustomCLIP_11407433138213 (moe_routing).

Math (per sample b with domain n = labels[b]):
    h   = relu(x @ W1[n])                 [R]
    a   = relu(h @ W2[n])                 [D]
    f   = 0.2*a + 0.8*x                   [D]
    out = exp(ls) * (f/||f||) @ T^T       [N_TXT]

v2 strategy (vs v1 which ran all 3 experts masked for every row):
  Host sorts rows by domain and shards them so each core gets 4 row
  blocks of 512: block 0 is a "flex" block covering at most 2 domains
  (one-hot masked, 2 expert slots), blocks 1-3 are single-domain "pure"
  blocks (1 expert slot, no masks).  The expert weights a block needs
  are gathered host-side into per-block weight tensors, so the device
  program is uniform across cores while each block runs only the
  expert(s) it needs.  PE work per core drops from 744 to 520 matmuls.

  Everything is computed transposed (samples on the free dim):
    - mm1: hT[s] = W1[s]^T XT per slot (PSUM, 8 K-chunks of 128).
    - g[s] = relu(hT[s]) (pure) or relu(hT[s]) * mask[s] (flex).
    - mm2: pa[d] = sum_s W2'[s]^T g[s], W2' = 0.25*W2 (folds 0.2/0.8).
    - fp[d] = relu(pa[d]) + XT[d]  == (0.2*a + 0.8*x)/0.8 per column.
    - s = colsum(fp^2) via ones-matmul; iv = 1/sqrt(s*exp(-2 ls));
      bcast(iv) via rank-1 matmul; logits scaled after mm3.
    - mm3: logitsT[t] = TT^T fp -> * bcast(iv) -> DRAM bf16.
  mm1/mm2/mm3 operands are bf16 (full PE rate, fp32 PSUM accumulate);
  the norm chain stays fp32(r).  Host inverse-permutes the output.
  Loads are batched into a few large DMAs from host-prearranged
  layouts (one 2D copy each) to cut DMA-trigger serialization.
"""

import contextlib
import os
import sys

sys.path.insert(0, "/opt/trn_rl_repo")

import ml_dtypes
import numpy as np

import concourse.bass as bass  # noqa: F401  (registers engine types)
import concourse.mybir as mybir
import concourse.tile as tile
from concourse import bacc
from concourse.bass_utils import run_bass_kernel_spmd

# Problem constants (hardcoded per task contract).
B, D, R, ND, NT = 16384, 1024, 256, 3, 1380
NC = 8                    # cores
BPC = B // NC             # rows per core = 2048
RB = 512                  # row-block (matmul moving dim)
NB = BPC // RB            # row-blocks per core = 4
KD = D // 128             # 8 contraction chunks over D
KR = R // 128             # 2 chunks over R
MR = R // 128             # 2 M-chunks over R
NTP = 1408                # text padded to 11*128
TTI = NTP // 128          # 11 text chunks
FLEX = 0                  # block slot with 2 expert slots
PURE_T = NC * (NB - 1)    # total pure blocks = 24

F32 = mybir.dt.float32
F32R = mybir.dt.float32r
BF16 = mybir.dt.bfloat16
F8 = mybir.dt.float8e4
MM_DT = BF16
NPDT = ml_dtypes.bfloat16
NPF8 = ml_dtypes.float8_e4m3fn
# fp8 scale plan: W1*64, W2*64 -> psum = 4096*(h@W2); residual x *16384
# makes fp_scaled = 16384*(0.25*relu(a) + x); the norm cancels the scale.
W_SCALE = 64.0
XR_SCALE = 16384.0


def _f32(ap):
    """View a matmul-dtype AP as plain fp32 for ACT/DVE consumption."""
    return ap.bitcast(F32) if MM_DT == F32R else ap


def _dr(ap):
    """Split an AP's free dim into the 2 k-tiles DoubleRow matmul expects."""
    return ap.rearrange("p (k n) -> p k n", k=2)


def build_program():
    nc = bacc.Bacc(
        "TRN2",
        target_bir_lowering=False,
        debug=False,
        enable_asserts=True,
        num_devices=NC,
    )
    xb = nc.declare_dram_parameter("xb", [NB, 128, KD * RB], F8, isOutput=False)
    xr = nc.declare_dram_parameter("xr", [NB, 128, KD * RB], BF16, isOutput=False)
    w1b = nc.declare_dram_parameter("w1b", [NB, 128, KD * R], F8, isOutput=False)
    w2b = nc.declare_dram_parameter("w2b", [NB, 128, KR * D], F8, isOutput=False)
    w1q = nc.declare_dram_parameter("w1q", [128, KD * R], F8, isOutput=False)
    w2q = nc.declare_dram_parameter("w2q", [128, KR * D], F8, isOutput=False)
    mk = nc.declare_dram_parameter("mk", [2, RB], BF16, isOutput=False)
    tt = nc.declare_dram_parameter("tt", [128, KD * NTP], MM_DT, isOutput=False)
    sc = nc.declare_dram_parameter("sc", [1, 1], F32, isOutput=False)
    oc = nc.declare_dram_parameter("oc", [128, 1], F32R, isOutput=False)
    orow = nc.declare_dram_parameter("orow", [1, 128], F32R, isOutput=False)
    ot = nc.declare_dram_parameter("ot", [NTP, BPC], BF16, isOutput=True)

    with tile.TileContext(nc) as tc, contextlib.ExitStack() as ctx:
        cst = ctx.enter_context(tc.tile_pool(name="cst", bufs=1))
        p_xb = ctx.enter_context(tc.tile_pool(name="p_xb", bufs=2))
        p_xr = ctx.enter_context(tc.tile_pool(name="p_xr", bufs=2))
        p_w1 = ctx.enter_context(tc.tile_pool(name="p_w1", bufs=2))
        p_w2 = ctx.enter_context(tc.tile_pool(name="p_w2", bufs=2))
        p_g = ctx.enter_context(tc.tile_pool(name="p_g", bufs=6))
        p_fp = ctx.enter_context(tc.tile_pool(name="p_fp", bufs=16))
        p_sq = ctx.enter_context(tc.tile_pool(name="p_sq", bufs=2))
        p_acc = ctx.enter_context(tc.tile_pool(name="p_acc", bufs=4))
        p_pbs = ctx.enter_context(tc.tile_pool(name="p_pbs", bufs=2))
        p_ob = ctx.enter_context(tc.tile_pool(name="p_ob", bufs=4))
        p_nrm = ctx.enter_context(tc.tile_pool(name="p_nrm", bufs=2))

        ps_h = ctx.enter_context(tc.tile_pool(name="ps_h", bufs=2, space="PSUM"))
        ps_a = ctx.enter_context(tc.tile_pool(name="ps_a", bufs=3, space="PSUM"))
        ps_s = ctx.enter_context(tc.tile_pool(name="ps_s", bufs=1, space="PSUM"))
        ps_l = ctx.enter_context(tc.tile_pool(name="ps_l", bufs=2, space="PSUM"))

        # ---- constant tiles -------------------------------------------------
        ttt = cst.tile([128, KD * NTP], MM_DT, name="ttt", tag="ttt")
        ones_col = cst.tile([128, 1], F32R, name="ones_col", tag="ones_col")
        ones_row = cst.tile([1, 128], F32R, name="ones_row", tag="ones_row")
        sct = cst.tile([1, 1], F32, name="sct", tag="sct")
        w1qt = cst.tile([128, KD * R], F8, name="w1qt", tag="w1qt")
        w2qt = cst.tile([128, KR * D], F8, name="w2qt", tag="w2qt")
        mka = cst.tile([128, RB], BF16, name="mka", tag="mka")
        mkb = cst.tile([128, RB], BF16, name="mkb", tag="mkb")

        S = [dict() for _ in range(NB)]

        def emit_cst():
            nc.sync.dma_start(sct[:], sc[:])
            nc.sync.dma_start(ones_col[:], oc[:])
            nc.sync.dma_start(ones_row[:], orow[:])

        def emit_tt():
            # on the scalar queue AFTER block-0/1 loads so the big text
            # matrix doesn't compete with the latency-critical prologue
            for i in range(4):
                w = 2 * NTP
                nc.scalar.dma_start(ttt[:, i * w : (i + 1) * w], tt[:, i * w : (i + 1) * w])

        def emit_loads(b):
            w1t = p_w1.tile([128, KD * R], F8, name="w1t", tag="w1t")
            nc.scalar.dma_start(w1t[:], w1b[b, :, :])
            xbig = p_xb.tile([128, KD * RB], F8, name="xbig", tag="xbig")
            q = 2 * RB  # first k-pair lands early so mm1 starts sooner
            nc.scalar.dma_start(xbig[:, :q], xb[b, :, :q])
            nc.scalar.dma_start(xbig[:, q:], xb[b, :, q:])
            if b == FLEX:
                nc.scalar.dma_start(w1qt[:], w1q[:])
            w2t = p_w2.tile([128, KR * D], F8, name="w2t", tag="w2t")
            nc.scalar.dma_start(w2t[:], w2b[b, :, :])
            if b == FLEX:
                nc.scalar.dma_start(w2qt[:], w2q[:])
                nc.scalar.dma_start(
                    mka[:],
                    mk[0, :].rearrange("(a n) -> a n", a=1).to_broadcast((128, RB)),
                )
                nc.scalar.dma_start(
                    mkb[:],
                    mk[1, :].rearrange("(a n) -> a n", a=1).to_broadcast((128, RB)),
                )
            xres = p_xr.tile([128, KD * RB], BF16, name="xres", tag="xres")
            half = KD * RB // 2
            nc.scalar.dma_start(xres[:, :half], xr[b, :, :half])
            nc.scalar.dma_start(xres[:, half:], xr[b, :, half:])
            S[b]["xbig"] = xbig
            S[b]["xres"] = xres
            S[b]["w1t"] = w1t
            S[b]["w2t"] = w2t

        DRM = mybir.MatmulPerfMode.DoubleRow

        def emit_mm1_g(b):
            xbig = S[b]["xbig"]
            slots = [(S[b]["w1t"], None)]
            if b == FLEX:
                slots = [(S[b]["w1t"], mka), (w1qt, mkb)]
            g = []
            for s, (wt, msk) in enumerate(slots):
                # one g tile per slot holding both r-chunks side by side
                # (the two k-tiles DoubleRow mm2 consumes)
                gt = p_g.tile([128, 2 * RB], F8, name="g", tag="g")
                for m in range(MR):
                    ph = ps_h.tile([128, RB], F32, name="ph", tag="ph")
                    for i in range(KD // 2):
                        nc.tensor.matmul(
                            ph[:],
                            _dr(wt[:, i * 2 * R + m * 2 * 128 : i * 2 * R + (m + 1) * 2 * 128]),
                            _dr(xbig[:, 2 * i * RB : (2 * i + 2) * RB]),
                            start=(i == 0),
                            stop=(i == KD // 2 - 1),
                            perf_mode=DRM,
                        )
                    gd = gt[:, m * RB : (m + 1) * RB]
                    if msk is None:
                        nc.vector.tensor_scalar_max(gd, ph[:], 0.0)
                    else:
                        nc.vector.scalar_tensor_tensor(
                            gd,
                            ph[:],
                            0.0,
                            msk[:],
                            mybir.AluOpType.max,
                            mybir.AluOpType.mult,
                        )
                g.append(gt)
            S[b]["g"] = g

        def emit_mm2(b):
            xres = S[b]["xres"]
            g = S[b]["g"]
            wts = [S[b]["w2t"]]
            if b == FLEX:
                wts = [S[b]["w2t"], w2qt]
            nmm = len(wts)
            fp = []
            acc = None
            for d in range(KD):
                pa = ps_a.tile([128, RB], F32, name="pa", tag="pa")
                for s, wt in enumerate(wts):
                    nc.tensor.matmul(
                        pa[:],
                        _dr(wt[:, d * 256 : (d + 1) * 256]),
                        _dr(g[s][:]),
                        start=(s == 0),
                        stop=(s == nmm - 1),
                        perf_mode=DRM,
                    )
                ft = p_fp.tile([128, RB], MM_DT, name="fp", tag="fp")
                nc.vector.scalar_tensor_tensor(
                    ft[:],
                    pa[:],
                    0.0,
                    xres[:, d * RB : (d + 1) * RB],
                    mybir.AluOpType.max,
                    mybir.AluOpType.add,
                )
                fp.append(ft)
                if d == 0:
                    acc = p_acc.tile([128, RB], F32, name="acc", tag="acc")
                    nc.scalar.square(acc[:], _f32(ft[:]))
                else:
                    st = p_sq.tile([128, RB], F32, name="sq", tag="sq")
                    nc.scalar.square(st[:], _f32(ft[:]))
                    if d < KD - 1:
                        nc.gpsimd.tensor_add(acc[:], acc[:], st[:])
                    else:
                        accm = p_acc.tile([128, RB], F32R, name="accm", tag="accm")
                        nc.gpsimd.tensor_add(accm[:], acc[:], st[:])
            S[b]["fp"] = fp
            S[b]["accm"] = accm

        def emit_ps_norm(b):
            accm = S[b]["accm"]
            ps = ps_s.tile([1, RB], F32, name="ps", tag="ps")
            nc.tensor.matmul(ps[:], ones_col[:], accm[:], start=True, stop=True)
            iv = p_nrm.tile([1, RB], F32R, name="iv", tag="iv")
            nc.scalar.activation(
                iv[:],
                ps[:],
                mybir.ActivationFunctionType.Abs_reciprocal_sqrt,
                scale=sct[:],
            )
            S[b]["iv"] = iv

        def emit_pb(b):
            iv = S[b]["iv"]
            pb = ps_l.tile([128, RB], F32, name="pl", tag="pl")
            nc.tensor.matmul(pb[:], ones_row[:], iv[:], start=True, stop=True)
            pbs = p_pbs.tile([128, RB], F32, name="pbs", tag="pbs")
            nc.scalar.copy(pbs[:], pb[:])
            S[b]["pbs"] = pbs

        def emit_mm3(b):
            c0 = b * RB
            fp = S[b]["fp"]
            pbs = S[b]["pbs"]
            for t_i in range(TTI):
                pl = ps_l.tile([128, RB], F32, name="pl", tag="pl")
                for k in range(KD):
                    nc.tensor.matmul(
                        pl[:],
                        ttt[:, k * NTP + t_i * 128 : k * NTP + (t_i + 1) * 128],
                        fp[k][:],
                        start=(k == 0),
                        stop=(k == KD - 1),
                    )
                ob = p_ob.tile([128, RB], BF16, name="ob", tag="ob")
                nc.vector.tensor_mul(ob[:], pl[:], pbs[:])
                nc.sync.dma_start(
                    ot[t_i * 128 : (t_i + 1) * 128, c0 : c0 + RB], ob[:]
                )
            S[b].clear()

        # ---- emission schedule (software pipelined) -------------------------
        emit_cst()
        emit_loads(0)
        emit_mm1_g(0)
        emit_loads(1)
        emit_tt()
        emit_mm2(0)
        emit_ps_norm(0)
        for b in range(NB):
            if b + 1 < NB:
                emit_mm1_g(b + 1)
            emit_pb(b)
            if b + 2 < NB:
                emit_loads(b + 2)
            if b + 1 < NB:
                emit_mm2(b + 1)
            emit_mm3(b)
            if b + 1 < NB:
                emit_ps_norm(b + 1)

    nc.compile()
    return nc


_NC_CACHE = None


def _get_program():
    global _NC_CACHE
    if _NC_CACHE is None:
        _NC_CACHE = build_program()
    return _NC_CACHE


def _pack(labels):
    """Sort rows by domain; return per-core block descriptors.

    Each core gets NB blocks of RB rows: slot FLEX may span 2 domains
    (A/B with one-hot masks), the rest are single-domain.  Returns
    per-core lists of (domA, domB, maskA, maskB, row_idx[RB]).
    """
    labels = np.asarray(labels).astype(np.int64).ravel()
    order = np.argsort(labels, kind="stable")
    counts = np.bincount(labels, minlength=ND).astype(np.int64)
    p = counts // RB
    while p.sum() > PURE_T:
        p[int(np.argmax(p))] -= 1
    assert p.sum() == PURE_T, (counts, p)

    starts = np.concatenate([[0], np.cumsum(counts)])
    pure_blocks = []
    segs = []
    for n in range(ND):
        run = order[starts[n] : starts[n + 1]]
        for i in range(int(p[n])):
            pure_blocks.append((n, run[i * RB : (i + 1) * RB]))
        segs.append((n, run[int(p[n]) * RB :]))
    # order leftover segments (small, big, small) so no flex bin spans 3 doms
    segs.sort(key=lambda s: len(s[1]))
    seg_order = [segs[0], segs[2], segs[1]]
    flat_rows = np.concatenate([s[1] for s in seg_order])
    flat_doms = np.concatenate(
        [np.full(len(s[1]), s[0], dtype=np.int64) for s in seg_order]
    )
    assert flat_rows.shape[0] == NC * RB

    cores = []
    for c in range(NC):
        rows = flat_rows[c * RB : (c + 1) * RB]
        doms = flat_doms[c * RB : (c + 1) * RB]
        dA, dB = int(doms[0]), int(doms[-1])
        mA = (doms == dA).astype(np.float32)
        mB = (1.0 - mA) if dA != dB else np.zeros(RB, dtype=np.float32)
        assert np.all((doms == dA) | (doms == dB))
        blocks = [(dA, dB, mA, mB.astype(np.float32), rows)]
        for s in range(NB - 1):
            n, prows = pure_blocks[c * (NB - 1) + s]
            blocks.append((n, n, None, None, prows))
        cores.append(blocks)
    return cores


def make_in_maps(image_features, domain_labels, W1, W2, text_features, logit_scale):
    X = np.asarray(image_features, dtype=np.float32)
    W1 = np.asarray(W1, dtype=np.float32)
    W2 = np.asarray(W2, dtype=np.float32)
    T = np.asarray(text_features, dtype=np.float32)
    ls = float(np.asarray(logit_scale))

    cores = _pack(domain_labels)

    # per-domain weight layouts for direct 2D DMA (fp8 DoubleRow order)
    # w1: [p, i*512 + m*256 + j*128 + mc] = 64*W1[(2i+j)*128+p, m*128+mc]
    w1L = [
        np.ascontiguousarray(
            (W_SCALE * W1[n])
            .reshape(KD // 2, 2, 128, MR, 128)
            .transpose(2, 0, 3, 1, 4)
            .reshape(128, KD * R)
        ).astype(NPF8)
        for n in range(ND)
    ]
    # w2: [p, d*256 + j*128 + dc] = 64*W2[j*128+p, d*128+dc]
    w2L = [
        np.ascontiguousarray(
            (W_SCALE * W2[n])
            .reshape(KR, 128, KD, 128)
            .transpose(1, 2, 0, 3)
            .reshape(128, KR * D)
        ).astype(NPF8)
        for n in range(ND)
    ]
    Tpad = np.zeros((NTP, D), dtype=np.float32)
    Tpad[:NT] = T
    ttL = np.ascontiguousarray(
        Tpad.reshape(NTP, KD, 128).transpose(2, 1, 0).reshape(128, KD * NTP)
    ).astype(NPDT)
    scv = np.array([[np.exp(-2.0 * ls)]], dtype=np.float32)
    ocv = np.ones((128, 1), dtype=np.float32)
    orv = np.ones((1, 128), dtype=np.float32)

    in_maps = []
    perms = []
    for blocks in cores:
        xbs, w1s, w2s = [], [], []
        perm = []
        for dA, dB, mA, mB, rows in blocks:
            Xr = X[rows]  # [RB, D]
            xbs.append(
                Xr.reshape(RB, KD, 128).transpose(2, 1, 0).reshape(128, KD * RB)
            )
            w1s.append(w1L[dA])
            w2s.append(w2L[dA])
            perm.append(rows)
        dA0, dB0, mA0, mB0, _ = blocks[FLEX]
        xbf = np.ascontiguousarray(np.stack(xbs))
        in_maps.append(
            {
                "xb": xbf.astype(NPF8),
                "xr": (XR_SCALE * xbf).astype(NPDT),
                "w1b": np.ascontiguousarray(np.stack(w1s)),
                "w2b": np.ascontiguousarray(np.stack(w2s)),
                "w1q": w1L[dB0],
                "w2q": w2L[dB0],
                "mk": np.ascontiguousarray(np.stack([mA0, mB0])).astype(NPDT),
                "tt": ttL,
                "sc": scv,
                "oc": ocv,
                "orow": orv,
            }
        )
        perms.append(perm)
    return in_maps, perms


def kernel(image_features, domain_labels, W1, W2, text_features, logit_scale, **kw):
    in_maps, perms = make_in_maps(
        image_features, domain_labels, W1, W2, text_features, logit_scale
    )
    nc = _get_program()
    res = run_bass_kernel_spmd(nc, in_maps, list(range(NC)))

    out = np.empty((B, NT), dtype=np.float32)
    for c in range(NC):
        otc = np.asarray(res.results[c]["ot"]).astype(np.float32)
        for b in range(NB):
            out[perms[c][b], :] = otc[:NT, b * RB : (b + 1) * RB].T
    return out
